# revision 2
# baseline (speedup 1.0000x reference)
"""MetaMasker kernel for 8 Trainium2 NeuronCores.

Data-parallel over batch B=32 across 8 cores (4 batches per core), weights
replicated — every stage (dilated convs, GAT diag-attention, top-k threshold,
masking) is batch-independent.

The whole forward runs on-device via jax.pmap over the 8 NeuronCores.
The GAT stage only needs the *diagonal* of the softmax attention matrix
(einsum 'btnn,btnf->btnf'), so we compute softmax row statistics of the
[N,N] score matrix and keep just the diagonal coefficients instead of
materializing/applying the full attention tensor.
"""

import numpy as np
import jax
import jax.numpy as jnp

B, T, N, F = 32, 64, 256, 64
TOP_K = 0.2
N_CORES = 8
B_LOC = B // N_CORES


def _conv_block(x, Wc1, bc1, Wc2, bc2, Wc3, bc3):
    # x: [b, T, N, F] -> conv over time per node, dilations 1,2,3, SAME pad.
    # Implement as sum of shifted matmuls (exactly equivalent to
    # lax.conv_general_dilated with WIO weights and NWC data).
    xt = jnp.transpose(x, (0, 2, 1, 3))  # [b, N, T, F]

    def conv1d(W, b_, dil):
        # taps at offsets -dil, 0, +dil with weights W[0], W[1], W[2]
        out = xt @ W[1]
        pad = jnp.zeros(xt.shape[:2] + (dil, xt.shape[3]), xt.dtype)
        x_m = jnp.concatenate([pad, xt[:, :, :-dil, :]], axis=2)  # x[t-dil]
        x_p = jnp.concatenate([xt[:, :, dil:, :], pad], axis=2)   # x[t+dil]
        out = out + x_m @ W[0] + x_p @ W[2]
        return out + b_

    c = jax.nn.relu(conv1d(Wc1, bc1, 1) + conv1d(Wc2, bc2, 2)
                    + conv1d(Wc3, bc3, 3))
    xt = xt + c
    return jnp.transpose(xt, (0, 2, 1, 3))  # [b, T, N, F]


def _forward(x, Wc1, bc1, Wc2, bc2, Wc3, bc3, Wq, bq, Wk, bk, Wv, bv,
             Wd, bd, Wp, bp):
    bx = x.shape[0]
    x = _conv_block(x, Wc1, bc1, Wc2, bc2, Wc3, bc3)

    # --- GAT attention over nodes; only diag(A) is consumed ---
    Q = x @ Wq + bq
    K = x @ Wk + bk
    V = x @ Wv + bv
    scores = jnp.einsum("btnf,btmf->btnm", Q, K) / jnp.sqrt(jnp.float32(F))
    m = jnp.max(scores, axis=-1)                      # [b,T,N]
    z = jnp.sum(jnp.exp(scores - m[..., None]), axis=-1)
    diag = jnp.einsum("btnn->btn", scores)
    a_diag = jnp.exp(diag - m) / z                    # [b,T,N]
    g = a_diag[..., None] * V
    x = x + jax.nn.relu(g)

    # --- dense projection, inf-norm normalize, rescale ---
    x = x @ Wd + bd
    norm = jnp.max(jnp.abs(x), axis=-1, keepdims=True)
    x = x / norm
    x = 0.5 * (x + 1.0)

    # --- per-batch top-k threshold (exact (n-k)-th smallest) ---
    # XLA-Neuron has no sort; find the order statistic by bisection on the
    # value. All values lie in [0, 1]. After 60 halvings the bracket [lo, hi)
    # is narrower than one float32 ulp, so every element in it equals lo and
    # lo IS the exact order statistic (invariant: count(y<lo) <= r < count(y<hi)).
    n = T * N * F
    k = int(round(TOP_K * n))
    r = jnp.float32(n - k - 1)  # 0-based rank of mu in ascending order
    y = x.reshape(bx, -1)

    def step(_, state):
        lo, hi = state
        mid = 0.5 * (lo + hi)
        cnt = jnp.sum((y < mid[:, None]).astype(jnp.float32), axis=-1)
        go_up = cnt <= r
        lo = jnp.where(go_up, mid, lo)
        hi = jnp.where(go_up, hi, mid)
        return lo, hi

    lo0 = jnp.zeros((bx,), jnp.float32)
    hi0 = jnp.ones((bx,), jnp.float32) * jnp.float32(1.0000001)
    lo, hi = jax.lax.fori_loop(0, 60, step, (lo0, hi0))
    mu = lo.reshape(bx, 1, 1, 1)
    x = jax.nn.sigmoid(x - mu)

    mask = jnp.round(x)
    masked = x * mask
    pred = masked @ Wp + bp
    return pred, mask


_pmapped = jax.pmap(_forward, axis_name="core")

_W_NAMES = ["Wc1", "bc1", "Wc2", "bc2", "Wc3", "bc3", "Wq", "bq", "Wk", "bk",
            "Wv", "bv", "Wd", "bd", "Wp", "bp"]


def kernel(**inputs):
    x = np.asarray(inputs["x"], dtype=np.float32)
    xs = x.reshape(N_CORES, B_LOC, T, N, F)
    ws = [np.broadcast_to(np.asarray(inputs[name], dtype=np.float32),
                          (N_CORES,) + np.asarray(inputs[name]).shape)
          for name in _W_NAMES]
    pred, mask = _pmapped(xs, *ws)
    pred = np.asarray(jax.device_get(pred)).reshape(B, T, N, F)
    mask = np.asarray(jax.device_get(mask)).reshape(B, T, N, F)
    return pred, mask


if __name__ == "__main__":
    import time
    rng = np.random.default_rng(0)
    demo = {"x": rng.standard_normal((B, T, N, F), dtype=np.float32)}
    for name in _W_NAMES:
        shape = (3, F, F) if name.startswith("Wc") else ((F, F) if name.startswith("W") else (F,))
        demo[name] = (rng.standard_normal(shape, dtype=np.float32) * 0.05
                      if name.startswith("W") else np.zeros(shape, np.float32))
    t0 = time.time(); out = kernel(**demo); t1 = time.time()
    print("first call (compile+run):", t1 - t0, "s")
    t0 = time.time(); out = kernel(**demo); t1 = time.time()
    print("steady call:", t1 - t0, "s", out[0].shape, out[1].shape)


# revision 4
# speedup vs baseline: 1.5531x; 1.5531x over previous
"""MetaMasker kernel for 8 Trainium2 NeuronCores.

Data-parallel over batch B=32 across 8 cores (4 batches per core), weights
replicated — every stage (dilated convs, GAT diag-attention, top-k threshold,
masking) is batch-independent.

The whole forward runs on-device via jax.pmap over the 8 NeuronCores.
The GAT stage only needs the *diagonal* of the softmax attention matrix
(einsum 'btnn,btnf->btnf'), so we compute softmax row statistics of the
[N,N] score matrix and keep just the diagonal coefficients instead of
materializing/applying the full attention tensor.
"""

import numpy as np
import jax
import jax.numpy as jnp

B, T, N, F = 32, 64, 256, 64
TOP_K = 0.2
N_CORES = 8
B_LOC = B // N_CORES


def _conv_block(x, Wc1, bc1, Wc2, bc2, Wc3, bc3):
    # x: [b, T, N, F] -> conv over time per node, dilations 1,2,3, SAME pad.
    # Implement as sum of shifted matmuls (exactly equivalent to
    # lax.conv_general_dilated with WIO weights and NWC data).
    xt = jnp.transpose(x, (0, 2, 1, 3))  # [b, N, T, F]

    def conv1d(W, b_, dil):
        # taps at offsets -dil, 0, +dil with weights W[0], W[1], W[2]
        out = xt @ W[1]
        pad = jnp.zeros(xt.shape[:2] + (dil, xt.shape[3]), xt.dtype)
        x_m = jnp.concatenate([pad, xt[:, :, :-dil, :]], axis=2)  # x[t-dil]
        x_p = jnp.concatenate([xt[:, :, dil:, :], pad], axis=2)   # x[t+dil]
        out = out + x_m @ W[0] + x_p @ W[2]
        return out + b_

    c = jax.nn.relu(conv1d(Wc1, bc1, 1) + conv1d(Wc2, bc2, 2)
                    + conv1d(Wc3, bc3, 3))
    xt = xt + c
    return jnp.transpose(xt, (0, 2, 1, 3))  # [b, T, N, F]


def _forward(x, Wc1, bc1, Wc2, bc2, Wc3, bc3, Wq, bq, Wk, bk, Wv, bv,
             Wd, bd, Wp, bp):
    bx = x.shape[0]
    x = _conv_block(x, Wc1, bc1, Wc2, bc2, Wc3, bc3)

    # --- GAT attention over nodes; only diag(A) is consumed ---
    Q = x @ Wq + bq
    K = x @ Wk + bk
    V = x @ Wv + bv
    scores = jnp.einsum("btnf,btmf->btnm", Q, K) / jnp.sqrt(jnp.float32(F))
    m = jnp.max(scores, axis=-1)                      # [b,T,N]
    z = jnp.sum(jnp.exp(scores - m[..., None]), axis=-1)
    diag = jnp.einsum("btnn->btn", scores)
    a_diag = jnp.exp(diag - m) / z                    # [b,T,N]
    g = a_diag[..., None] * V
    x = x + jax.nn.relu(g)

    # --- dense projection, inf-norm normalize, rescale ---
    x = x @ Wd + bd
    norm = jnp.max(jnp.abs(x), axis=-1, keepdims=True)
    x = x / norm
    x = 0.5 * (x + 1.0)

    # --- per-batch top-k threshold (exact (n-k)-th smallest) ---
    # XLA-Neuron has no sort; find the order statistic by bisection on the
    # value. All values lie in [0, 1]. After 60 halvings the bracket [lo, hi)
    # is narrower than one float32 ulp, so every element in it equals lo and
    # lo IS the exact order statistic (invariant: count(y<lo) <= r < count(y<hi)).
    n = T * N * F
    k = int(round(TOP_K * n))
    r = jnp.float32(n - k - 1)  # 0-based rank of mu in ascending order
    y = x.reshape(bx, -1)

    def step(_, state):
        lo, hi = state
        mid = 0.5 * (lo + hi)
        cnt = jnp.sum((y < mid[:, None]).astype(jnp.float32), axis=-1)
        go_up = cnt <= r
        lo = jnp.where(go_up, mid, lo)
        hi = jnp.where(go_up, hi, mid)
        return lo, hi

    lo0 = jnp.zeros((bx,), jnp.float32)
    hi0 = jnp.ones((bx,), jnp.float32) * jnp.float32(1.0000001)
    # 36 halvings reach float32-ulp adjacency for any mu >= 2^-12; mu is the
    # 80th percentile of values in [0,1] with >= 1 element per token at 1.0,
    # so it is far above that.
    lo, hi = jax.lax.fori_loop(0, 36, step, (lo0, hi0))
    mu = lo.reshape(bx, 1, 1, 1)
    x = jax.nn.sigmoid(x - mu)

    # round-half-even of sigmoid in (0,1) is exactly (x > 0.5)
    mask = (x > 0.5).astype(jnp.float32)
    masked = x * mask
    pred = masked @ Wp + bp
    # mask is exactly 0/1 -> bf16 wire format halves device->host traffic
    return pred, mask.astype(jnp.bfloat16)


_pmapped = jax.pmap(_forward, axis_name="core")

_W_NAMES = ["Wc1", "bc1", "Wc2", "bc2", "Wc3", "bc3", "Wq", "bq", "Wk", "bk",
            "Wv", "bv", "Wd", "bd", "Wp", "bp"]


def kernel(**inputs):
    x = np.asarray(inputs["x"], dtype=np.float32)
    xs = x.reshape(N_CORES, B_LOC, T, N, F)
    ws = [np.broadcast_to(np.asarray(inputs[name], dtype=np.float32),
                          (N_CORES,) + np.asarray(inputs[name]).shape)
          for name in _W_NAMES]
    pred, mask = _pmapped(xs, *ws)
    pred = np.asarray(jax.device_get(pred)).reshape(B, T, N, F)
    mask = np.asarray(jax.device_get(mask)).astype(np.float32).reshape(B, T, N, F)
    return pred, mask


if __name__ == "__main__":
    import time
    rng = np.random.default_rng(0)
    demo = {"x": rng.standard_normal((B, T, N, F), dtype=np.float32)}
    for name in _W_NAMES:
        shape = (3, F, F) if name.startswith("Wc") else ((F, F) if name.startswith("W") else (F,))
        demo[name] = (rng.standard_normal(shape, dtype=np.float32) * 0.05
                      if name.startswith("W") else np.zeros(shape, np.float32))
    t0 = time.time(); out = kernel(**demo); t1 = time.time()
    print("first call (compile+run):", t1 - t0, "s")
    t0 = time.time(); out = kernel(**demo); t1 = time.time()
    print("steady call:", t1 - t0, "s", out[0].shape, out[1].shape)


# revision 5
# speedup vs baseline: 108.1573x; 69.6407x over previous
"""MetaMasker kernel for 8 Trainium2 NeuronCores.

Data-parallel over batch B=32 across 8 cores (4 batches per core), weights
replicated — every stage (dilated convs, GAT diag-attention, top-k threshold,
masking) is batch-independent.

The whole forward runs on-device via jax.pmap over the 8 NeuronCores.
The GAT stage only needs the *diagonal* of the softmax attention matrix
(einsum 'btnn,btnf->btnf'), so we compute softmax row statistics of the
[N,N] score matrix and keep just the diagonal coefficients instead of
materializing/applying the full attention tensor.
"""

import numpy as np
import jax
import jax.numpy as jnp

B, T, N, F = 32, 64, 256, 64
TOP_K = 0.2
N_CORES = 8
B_LOC = B // N_CORES


def _conv_block(x, Wc1, bc1, Wc2, bc2, Wc3, bc3):
    # x: [b, T, N, F] -> conv over time per node, dilations 1,2,3, SAME pad.
    # Sum of three dilated 3-tap convs == one 7-tap conv with taps at offsets
    # -3..3; the three center taps share one combined weight. Implemented as
    # shifted matmuls (exactly equivalent to lax.conv_general_dilated with
    # WIO weights and NWC data, up to fp32 summation order).
    xt = jnp.transpose(x, (0, 2, 1, 3))  # [b, N, T, F]
    w_center = Wc1[1] + Wc2[1] + Wc3[1]
    taps = [(-1, Wc1[0]), (1, Wc1[2]), (-2, Wc2[0]), (2, Wc2[2]),
            (-3, Wc3[0]), (3, Wc3[2])]

    out = xt @ w_center
    for off, W in taps:
        d = abs(off)
        pad = jnp.zeros(xt.shape[:2] + (d, xt.shape[3]), xt.dtype)
        if off < 0:
            xs = jnp.concatenate([pad, xt[:, :, :-d, :]], axis=2)  # x[t-d]
        else:
            xs = jnp.concatenate([xt[:, :, d:, :], pad], axis=2)   # x[t+d]
        out = out + xs @ W

    c = jax.nn.relu(out + (bc1 + bc2 + bc3))
    xt = xt + c
    return jnp.transpose(xt, (0, 2, 1, 3))  # [b, T, N, F]


def _forward(x, Wc1, bc1, Wc2, bc2, Wc3, bc3, Wq, bq, Wk, bk, Wv, bv,
             Wd, bd, Wp, bp):
    bx = x.shape[0]
    x = _conv_block(x, Wc1, bc1, Wc2, bc2, Wc3, bc3)

    # --- GAT attention over nodes; only diag(A) is consumed ---
    Q = x @ Wq + bq
    K = x @ Wk + bk
    V = x @ Wv + bv
    scores = jnp.einsum("btnf,btmf->btnm", Q, K) / jnp.sqrt(jnp.float32(F))
    m = jnp.max(scores, axis=-1)                      # [b,T,N]
    z = jnp.sum(jnp.exp(scores - m[..., None]), axis=-1)
    diag = jnp.einsum("btnn->btn", scores)
    a_diag = jnp.exp(diag - m) / z                    # [b,T,N]
    g = a_diag[..., None] * V
    x = x + jax.nn.relu(g)

    # --- dense projection, inf-norm normalize, rescale ---
    x = x @ Wd + bd
    norm = jnp.max(jnp.abs(x), axis=-1, keepdims=True)
    x = x / norm
    x = 0.5 * (x + 1.0)

    # --- per-batch top-k threshold (exact (n-k)-th smallest) ---
    # XLA-Neuron has no sort; find the order statistic by bisection on the
    # value. All values lie in [0, 1]. After 60 halvings the bracket [lo, hi)
    # is narrower than one float32 ulp, so every element in it equals lo and
    # lo IS the exact order statistic (invariant: count(y<lo) <= r < count(y<hi)).
    n = T * N * F
    k = int(round(TOP_K * n))
    r = jnp.float32(n - k - 1)  # 0-based rank of mu in ascending order
    y = x.reshape(bx, -1)

    def step(_, state):
        lo, hi = state
        mid = 0.5 * (lo + hi)
        cnt = jnp.sum((y < mid[:, None]).astype(jnp.float32), axis=-1)
        go_up = cnt <= r
        lo = jnp.where(go_up, mid, lo)
        hi = jnp.where(go_up, hi, mid)
        return lo, hi

    lo0 = jnp.zeros((bx,), jnp.float32)
    hi0 = jnp.ones((bx,), jnp.float32) * jnp.float32(1.0000001)
    # 36 halvings reach float32-ulp adjacency for any mu >= 2^-12; mu is the
    # 80th percentile of values in [0,1] with >= 1 element per token at 1.0,
    # so it is far above that.
    lo, hi = jax.lax.fori_loop(0, 36, step, (lo0, hi0))
    mu = lo.reshape(bx, 1, 1, 1)
    x = jax.nn.sigmoid(x - mu)

    # round-half-even of sigmoid in (0,1) is exactly (x > 0.5)
    mask = (x > 0.5).astype(jnp.float32)
    masked = x * mask
    pred = masked @ Wp + bp
    # mask is exactly 0/1 -> bf16 wire format halves device->host traffic
    return pred, mask.astype(jnp.bfloat16)


_pmapped = jax.pmap(_forward, axis_name="core")

_W_NAMES = ["Wc1", "bc1", "Wc2", "bc2", "Wc3", "bc3", "Wq", "bq", "Wk", "bk",
            "Wv", "bv", "Wd", "bd", "Wp", "bp"]


def kernel(**inputs):
    x = np.asarray(inputs["x"], dtype=np.float32)
    xs = x.reshape(N_CORES, B_LOC, T, N, F)
    ws = [np.broadcast_to(np.asarray(inputs[name], dtype=np.float32),
                          (N_CORES,) + np.asarray(inputs[name]).shape)
          for name in _W_NAMES]
    pred, mask = _pmapped(xs, *ws)
    pred = np.asarray(jax.device_get(pred)).reshape(B, T, N, F)
    mask = np.asarray(jax.device_get(mask)).astype(np.float32).reshape(B, T, N, F)
    return pred, mask


if __name__ == "__main__":
    import time
    rng = np.random.default_rng(0)
    demo = {"x": rng.standard_normal((B, T, N, F), dtype=np.float32)}
    for name in _W_NAMES:
        shape = (3, F, F) if name.startswith("Wc") else ((F, F) if name.startswith("W") else (F,))
        demo[name] = (rng.standard_normal(shape, dtype=np.float32) * 0.05
                      if name.startswith("W") else np.zeros(shape, np.float32))
    t0 = time.time(); out = kernel(**demo); t1 = time.time()
    print("first call (compile+run):", t1 - t0, "s")
    t0 = time.time(); out = kernel(**demo); t1 = time.time()
    print("steady call:", t1 - t0, "s", out[0].shape, out[1].shape)


# revision 7
# speedup vs baseline: 130.2996x; 1.2047x over previous
"""MetaMasker kernel for 8 Trainium2 NeuronCores.

Data-parallel over batch B=32 across 8 cores (4 batches per core), weights
replicated — every stage (dilated convs, GAT diag-attention, top-k threshold,
masking) is batch-independent.

The whole forward runs on-device via jax.pmap over the 8 NeuronCores.
The GAT stage only needs the *diagonal* of the softmax attention matrix
(einsum 'btnn,btnf->btnf'), so we compute softmax row statistics of the
[N,N] score matrix and keep just the diagonal coefficients instead of
materializing/applying the full attention tensor.
"""

import numpy as np
import jax
import jax.numpy as jnp

B, T, N, F = 32, 64, 256, 64
TOP_K = 0.2
N_CORES = 8
B_LOC = B // N_CORES


def _conv_block(x, Wc1, bc1, Wc2, bc2, Wc3, bc3):
    # x: [b, T, N, F] -> conv over time per node, dilations 1,2,3, SAME pad.
    # Sum of three dilated 3-tap convs == one 7-tap conv with taps at offsets
    # -3..3; the three center taps share one combined weight. Implemented as
    # shifted matmuls (exactly equivalent to lax.conv_general_dilated with
    # WIO weights and NWC data, up to fp32 summation order).
    xt = jnp.transpose(x, (0, 2, 1, 3))  # [b, N, T, F]
    w_center = Wc1[1] + Wc2[1] + Wc3[1]
    taps = [(-1, Wc1[0]), (1, Wc1[2]), (-2, Wc2[0]), (2, Wc2[2]),
            (-3, Wc3[0]), (3, Wc3[2])]

    out = xt @ w_center
    for off, W in taps:
        d = abs(off)
        pad = jnp.zeros(xt.shape[:2] + (d, xt.shape[3]), xt.dtype)
        if off < 0:
            xs = jnp.concatenate([pad, xt[:, :, :-d, :]], axis=2)  # x[t-d]
        else:
            xs = jnp.concatenate([xt[:, :, d:, :], pad], axis=2)   # x[t+d]
        out = out + xs @ W

    c = jax.nn.relu(out + (bc1 + bc2 + bc3))
    xt = xt + c
    return jnp.transpose(xt, (0, 2, 1, 3))  # [b, T, N, F]


def _forward(x, Wc1, bc1, Wc2, bc2, Wc3, bc3, Wq, bq, Wk, bk, Wv, bv,
             Wd, bd, Wp, bp):
    bx = x.shape[0]
    x = _conv_block(x, Wc1, bc1, Wc2, bc2, Wc3, bc3)

    # --- GAT attention over nodes; only diag(A) is consumed ---
    Q = x @ Wq + bq
    K = x @ Wk + bk
    V = x @ Wv + bv
    scores = jnp.einsum("btnf,btmf->btnm", Q, K) / jnp.sqrt(jnp.float32(F))
    m = jnp.max(scores, axis=-1)                      # [b,T,N]
    z = jnp.sum(jnp.exp(scores - m[..., None]), axis=-1)
    diag = jnp.einsum("btnn->btn", scores)
    a_diag = jnp.exp(diag - m) / z                    # [b,T,N]
    g = a_diag[..., None] * V
    x = x + jax.nn.relu(g)

    # --- dense projection, inf-norm normalize, rescale ---
    x = x @ Wd + bd
    norm = jnp.max(jnp.abs(x), axis=-1, keepdims=True)
    x = x / norm
    x = 0.5 * (x + 1.0)

    # --- per-batch top-k threshold (exact (n-k)-th smallest) ---
    # XLA-Neuron has no sort; find the order statistic by bisection on the
    # value. All values lie in [0, 1]. After 60 halvings the bracket [lo, hi)
    # is narrower than one float32 ulp, so every element in it equals lo and
    # lo IS the exact order statistic (invariant: count(y<lo) <= r < count(y<hi)).
    n = T * N * F
    k = int(round(TOP_K * n))
    r = jnp.float32(n - k - 1)  # 0-based rank of mu in ascending order
    y = x.reshape(bx, -1)

    def step(_, state):
        lo, hi = state
        mid = 0.5 * (lo + hi)
        cnt = jnp.sum((y < mid[:, None]).astype(jnp.float32), axis=-1)
        go_up = cnt <= r
        lo = jnp.where(go_up, mid, lo)
        hi = jnp.where(go_up, hi, mid)
        return lo, hi

    lo0 = jnp.zeros((bx,), jnp.float32)
    hi0 = jnp.ones((bx,), jnp.float32) * jnp.float32(1.0000001)
    # 36 halvings reach float32-ulp adjacency for any mu >= 2^-12; mu is the
    # 80th percentile of values in [0,1] with >= 1 element per token at 1.0,
    # so it is far above that.
    lo, hi = jax.lax.fori_loop(0, 36, step, (lo0, hi0))
    mu = lo.reshape(bx, 1, 1, 1)
    x = jax.nn.sigmoid(x - mu)

    # round-half-even of sigmoid in (0,1) is exactly (x > 0.5)
    mask = (x > 0.5).astype(jnp.float32)
    masked = x * mask
    pred = masked @ Wp + bp
    # mask is exactly 0/1 -> bf16 wire format halves device->host traffic
    return pred, mask.astype(jnp.bfloat16)


_pmapped = jax.pmap(_forward, axis_name="core")

_W_NAMES = ["Wc1", "bc1", "Wc2", "bc2", "Wc3", "bc3", "Wq", "bq", "Wk", "bk",
            "Wv", "bv", "Wd", "bd", "Wp", "bp"]


def kernel(**inputs):
    x = np.asarray(inputs["x"], dtype=np.float32)
    xs = x.reshape(N_CORES, B_LOC, T, N, F)
    ws = [np.broadcast_to(np.asarray(inputs[name], dtype=np.float32),
                          (N_CORES,) + np.asarray(inputs[name]).shape)
          for name in _W_NAMES]
    pred, mask = _pmapped(xs, *ws)
    pred = np.asarray(jax.device_get(pred)).reshape(B, T, N, F)
    mask = np.asarray(jax.device_get(mask)).astype(np.float32).reshape(B, T, N, F)
    return pred, mask


if __name__ == "__main__":
    import time
    rng = np.random.default_rng(0)
    demo = {"x": rng.standard_normal((B, T, N, F), dtype=np.float32)}
    for name in _W_NAMES:
        shape = (3, F, F) if name.startswith("Wc") else ((F, F) if name.startswith("W") else (F,))
        demo[name] = (rng.standard_normal(shape, dtype=np.float32) * 0.05
                      if name.startswith("W") else np.zeros(shape, np.float32))
    t0 = time.time(); out = kernel(**demo); t1 = time.time()
    print("first call (compile+run):", t1 - t0, "s")
    t0 = time.time(); out = kernel(**demo); t1 = time.time()
    print("steady call:", t1 - t0, "s", out[0].shape, out[1].shape)


# revision 8
# speedup vs baseline: 166.2887x; 1.2762x over previous
"""MetaMasker kernel for 8 Trainium2 NeuronCores.

Data-parallel over batch B=32 across 8 cores (4 batches per core), weights
replicated — every stage (dilated convs, GAT diag-attention, top-k threshold,
masking) is batch-independent.

The whole forward runs on-device via jax.pmap over the 8 NeuronCores.
The GAT stage only needs the *diagonal* of the softmax attention matrix
(einsum 'btnn,btnf->btnf'), so we compute softmax row statistics of the
[N,N] score matrix and keep just the diagonal coefficients instead of
materializing/applying the full attention tensor.
"""

import numpy as np
import jax
import jax.numpy as jnp

B, T, N, F = 32, 64, 256, 64
TOP_K = 0.2
N_CORES = 8
B_LOC = B // N_CORES


def _conv_block(x, Wc1, bc1, Wc2, bc2, Wc3, bc3):
    # x: [b, T, N, F] -> conv over time per node, dilations 1,2,3, SAME pad.
    # Sum of three dilated 3-tap convs == one 7-tap conv with taps at offsets
    # -3..3; the three center taps share one combined weight. Implemented as
    # shifted matmuls (exactly equivalent to lax.conv_general_dilated with
    # WIO weights and NWC data, up to fp32 summation order).
    xt = jnp.transpose(x, (0, 2, 1, 3))  # [b, N, T, F]
    w_center = Wc1[1] + Wc2[1] + Wc3[1]
    taps = [(-1, Wc1[0]), (1, Wc1[2]), (-2, Wc2[0]), (2, Wc2[2]),
            (-3, Wc3[0]), (3, Wc3[2])]

    out = xt @ w_center
    for off, W in taps:
        d = abs(off)
        pad = jnp.zeros(xt.shape[:2] + (d, xt.shape[3]), xt.dtype)
        if off < 0:
            xs = jnp.concatenate([pad, xt[:, :, :-d, :]], axis=2)  # x[t-d]
        else:
            xs = jnp.concatenate([xt[:, :, d:, :], pad], axis=2)   # x[t+d]
        out = out + xs @ W

    c = jax.nn.relu(out + (bc1 + bc2 + bc3))
    xt = xt + c
    return jnp.transpose(xt, (0, 2, 1, 3))  # [b, T, N, F]


def _forward(x, Wc1, bc1, Wc2, bc2, Wc3, bc3, Wq, bq, Wk, bk, Wv, bv,
             Wd, bd, Wp, bp):
    bx = x.shape[0]
    x = _conv_block(x, Wc1, bc1, Wc2, bc2, Wc3, bc3)

    # --- GAT attention over nodes; only diag(A) is consumed ---
    Q = x @ Wq + bq
    K = x @ Wk + bk
    V = x @ Wv + bv
    scores = jnp.einsum("btnf,btmf->btnm", Q, K) / jnp.sqrt(jnp.float32(F))
    m = jnp.max(scores, axis=-1)                      # [b,T,N]
    z = jnp.sum(jnp.exp(scores - m[..., None]), axis=-1)
    diag = jnp.einsum("btnn->btn", scores)
    a_diag = jnp.exp(diag - m) / z                    # [b,T,N]
    g = a_diag[..., None] * V
    x = x + jax.nn.relu(g)

    # --- dense projection, inf-norm normalize, rescale ---
    x = x @ Wd + bd
    norm = jnp.max(jnp.abs(x), axis=-1, keepdims=True)
    x = x / norm
    x = 0.5 * (x + 1.0)

    # --- per-batch top-k threshold (exact (n-k)-th smallest) ---
    # XLA-Neuron has no sort; find the order statistic by bisection on the
    # value. All values lie in [0, 1]. After 60 halvings the bracket [lo, hi)
    # is narrower than one float32 ulp, so every element in it equals lo and
    # lo IS the exact order statistic (invariant: count(y<lo) <= r < count(y<hi)).
    n = T * N * F
    k = int(round(TOP_K * n))
    r = jnp.float32(n - k - 1)  # 0-based rank of mu in ascending order
    y = x.reshape(bx, -1)

    def step4(_, state):
        # 4 halvings per loop trip: amortizes the Neuron `while`-op
        # per-iteration overhead over 4 fused compare+count passes.
        lo, hi = state
        for _ in range(4):
            mid = 0.5 * (lo + hi)
            cnt = jnp.sum((y < mid[:, None]).astype(jnp.float32), axis=-1)
            go_up = cnt <= r
            lo = jnp.where(go_up, mid, lo)
            hi = jnp.where(go_up, hi, mid)
        return lo, hi

    lo0 = jnp.zeros((bx,), jnp.float32)
    hi0 = jnp.ones((bx,), jnp.float32) * jnp.float32(1.0000001)
    # 8x4 = 32 halvings reach float32-ulp adjacency for any mu >= 2^-9; mu is
    # the 80th percentile of values in [0,1] with >= 1 element per token at
    # 1.0, so it is far above that.
    lo, hi = jax.lax.fori_loop(0, 8, step4, (lo0, hi0))
    mu = lo.reshape(bx, 1, 1, 1)
    x = jax.nn.sigmoid(x - mu)

    # round-half-even of sigmoid in (0,1) is exactly (x > 0.5)
    mask = (x > 0.5).astype(jnp.float32)
    masked = x * mask
    pred = masked @ Wp + bp
    # mask is exactly 0/1 -> bf16 wire format halves device->host traffic
    return pred, mask.astype(jnp.bfloat16)


_pmapped = jax.pmap(_forward, axis_name="core")

_W_NAMES = ["Wc1", "bc1", "Wc2", "bc2", "Wc3", "bc3", "Wq", "bq", "Wk", "bk",
            "Wv", "bv", "Wd", "bd", "Wp", "bp"]


def kernel(**inputs):
    x = np.asarray(inputs["x"], dtype=np.float32)
    xs = x.reshape(N_CORES, B_LOC, T, N, F)
    ws = [np.broadcast_to(np.asarray(inputs[name], dtype=np.float32),
                          (N_CORES,) + np.asarray(inputs[name]).shape)
          for name in _W_NAMES]
    pred, mask = _pmapped(xs, *ws)
    pred = np.asarray(jax.device_get(pred)).reshape(B, T, N, F)
    mask = np.asarray(jax.device_get(mask)).astype(np.float32).reshape(B, T, N, F)
    return pred, mask


if __name__ == "__main__":
    import time
    rng = np.random.default_rng(0)
    demo = {"x": rng.standard_normal((B, T, N, F), dtype=np.float32)}
    for name in _W_NAMES:
        shape = (3, F, F) if name.startswith("Wc") else ((F, F) if name.startswith("W") else (F,))
        demo[name] = (rng.standard_normal(shape, dtype=np.float32) * 0.05
                      if name.startswith("W") else np.zeros(shape, np.float32))
    t0 = time.time(); out = kernel(**demo); t1 = time.time()
    print("first call (compile+run):", t1 - t0, "s")
    t0 = time.time(); out = kernel(**demo); t1 = time.time()
    print("steady call:", t1 - t0, "s", out[0].shape, out[1].shape)
